# revision 36
# baseline (speedup 1.0000x reference)
"""Binary conv + BN(train) + ReLU fused Trainium2 SPMD kernel.

Reference computation (NCHW, x:(32,256,56,56) f32):
    mean/var over (N,H,W) per channel; xn = (x-mean)*rsqrt(var+eps)*gamma+beta
    xb = sign(xn); wb = sign(W); y = relu(conv3x3(xb, wb, pad=1) + bias)

Strategy: data-parallel over batch across 8 NeuronCores (4 images each).
Per-core partial BN stats (bn_stats/bn_aggr on DVE, pipelined with the x
load) are exchanged via direct core-to-core RDMA broadcasts (XOR-relative
destinations, one 16B/partition payload per core; remote semaphores count
arrivals) instead of a CC AllReduce, cutting ~50us of collective latency.
Normalize+sign runs as scalar-engine activations Sign(a*x+b) writing fp8
into zero-padded 58x58 planes; the 3x3 conv is 9 accumulating DoubleRow
fp8 matmuls (K=256 via the paired-row mode) per 128x448 output tile with
weights loaded once per (block, tap); bias+relu drains on the vector
engine (tensor_scalar add+max), keeping the scalar engine free for sign.
Dummy matmuls gated on stats-readiness warm the PE clock (HAM) during the
exchange. Sign values are exact in fp8 and PSUM accumulates in fp32, so
the binarized conv is exact.
"""

import sys

for _p in ("/opt/trn_rl_repo", "/root/.axon_site/_ro/trn_rl_repo"):
    if _p not in sys.path:
        sys.path.append(_p)

import numpy as np

import concourse.bass as bass
from concourse.bass import ds
import concourse.mybir as mybir
import concourse.tile as tile
from concourse import bacc, bass_utils

F32 = mybir.dt.float32
FP8 = mybir.dt.float8e4
AF = mybir.ActivationFunctionType
ALU = mybir.AluOpType

N_CORES = 8
NB = 4          # images per core
C = 256
P = 128         # partitions / chunk size
NCH = 2         # channel chunks (ci and co)
H = W = 56
HW = H * W      # 3136
PH = PW = 58    # padded plane
PSZ = PH * PW   # 3364
RG = 8          # output rows per psum tile
NG = H // RG    # 7 row groups
NT = RG * W     # 448 columns per matmul
BN_EPS = 1e-5
BLK = 8         # psum tiles in flight per weight-reuse block

USE_RDMA = False  # direct RDMA stats exchange instead of CC AllReduce
USE_AG = True      # AllGather + local sum instead of AllReduce
SIM_LIGHT = False  # skip conv+warmup (sim can't model 4D DoubleRow matmuls)
DBG = False        # dump exchange tiles to the dbg output
DK_WARM = 120     # dummy matmuls to warm the PE during the exchange

_CACHE = {}


def _emit_warmup(nc, psum, wdum, wb):
    """PE warmup: zero matmuls gated on wdum (written at exchange-trigger
    time) keep the HAM activity window busy so the real conv starts at full
    clock. Results land in a pool psum tile that is never read."""
    dums = psum.tile([P, NT], F32, name="dums", tag="ps")
    for _ in range(DK_WARM):
        nc.tensor.matmul(
            dums[:], wdum[:], wb[:, :, 0:NT],
            start=True, stop=True,
            perf_mode=mybir.MatmulPerfMode.DoubleRow,
        )


def _build_nc():
    nc = bacc.Bacc("TRN2", target_bir_lowering=False, debug=False,
                   num_devices=N_CORES, num_swdge_queues=3)
    xs = nc.dram_tensor("xs", [NB, C, H, W], F32, kind="ExternalInput")
    wt = nc.dram_tensor("wt", [P, NCH, 9 * NCH * P], FP8, kind="ExternalInput")
    par = nc.dram_tensor("par", [NCH, P, 3], F32, kind="ExternalInput")
    ys = nc.dram_tensor("ys", [NB, C, H, W], F32, kind="ExternalOutput")
    dbg = (nc.dram_tensor("dbg", [P, 28], F32, kind="ExternalOutput")
           if (SIM_LIGHT or DBG) else None)

    with tile.TileContext(nc) as tc:
        with (
            tc.tile_pool(name="main", bufs=1) as main,
            tc.tile_pool(name="outp", bufs=4) as outp,
            tc.tile_pool(name="psum", bufs=8, space="PSUM") as psum,
            tc.tile_pool(name="dram", bufs=1, space="DRAM") as dram,
        ):
            xt = [main.tile([P, NB * HW], F32, name=f"xt{c}") for c in range(NCH)]
            # sign planes: [p, ci_chunk, image, padded 58x58] (chunk dim = fp8
            # DoubleRow pair dim)
            xball = main.tile([P, NCH, NB * PSZ], FP8, name="xball")
            xbv = xball.rearrange("p j (n h w) -> p j n h w", n=NB, h=PH)
            wb = main.tile([P, NCH, 9 * NCH * P], FP8, name="wb")
            parc = main.tile([P, 3 * NCH], F32, name="parc")  # [gamma,beta,bias] x chunk
            st6 = [main.tile([P, NB * 7 * 6], F32, name=f"st6{c}") for c in range(NCH)]
            tx0 = main.tile([P, 4], F32, name="tx0")      # [m0,e0,m1,e1]/8 payload
            rxb = main.tile([P, 3, 4], F32, name="rxb")   # butterfly recv slots
            s1 = main.tile([P, 4], F32, name="s1")
            s2 = main.tile([P, 4], F32, name="s2")
            tscr = main.tile([P, 4], F32, name="tscr")
            dbt = main.tile([P, 28], F32, name="dbt")
            wdum = main.tile([P, NCH, P], FP8, name="wdum")
            scr_a = main.tile([P, 1], F32, name="scr_a")
            scr_b = main.tile([P, 1], F32, name="scr_b")

            # load x (channels on partitions): chunk 0 from ScalarE's HWDGE
            # queue, chunk 1 from Sync's, so ring configs overlap and the load
            # starts as early as either engine clears its preamble
            HH = HW // 2
            for n in range(NB):
                for h in range(2):
                    nc.scalar.dma_start(
                        xt[0][:, n * HW + h * HH:n * HW + (h + 1) * HH],
                        xs[n, 0:P].rearrange("p h w -> p (h w)")[:, h * HH:(h + 1) * HH],
                    )
                    nc.sync.dma_start(
                        xt[1][:, n * HW + h * HH:n * HW + (h + 1) * HH],
                        xs[n, P:2 * P].rearrange("p h w -> p (h w)")[:, h * HH:(h + 1) * HH],
                    )
            nc.scalar.dma_start(wb[:], wt[:])
            nc.sync.dma_start(
                parc.rearrange("p (c s) -> p c s", s=3),
                par.rearrange("c p s -> p c s"),
            )

            # activation-table preload: pull sqrt_and_friends (covers Sqrt/
            # Sign/Relu) into the scalar engine during the x load so no
            # ACT_TABLE_LOAD lands on the post-exchange critical path
            nc.gpsimd.memset(scr_a[:], 1.0)
            nc.scalar.activation(scr_b[:], scr_a[:], AF.Sqrt)
            nc.scalar.activation(scr_b[:], scr_a[:], AF.Sign)
            nc.scalar.activation(scr_b[:], scr_a[:], AF.Relu)

            # zero only the pad borders of the sign planes (GpSimd; interior
            # is fully overwritten by the Sign activation)
            nc.gpsimd.memset(tx0[:], 0.0)
            for c in range(NCH):
                for n in range(NB):
                    nc.gpsimd.memset(xbv[:, c, n, 0, :], 0.0)
                    nc.gpsimd.memset(xbv[:, c, n, PH - 1, :], 0.0)
                    nc.gpsimd.memset(xbv[:, c, n, 1:PH - 1, 0], 0.0)
                    nc.gpsimd.memset(xbv[:, c, n, 1:PH - 1, PW - 1], 0.0)

            if USE_RDMA:
                # hypercube butterfly all-reduce of the 2KB stats payload in
                # 3 rounds of XOR-relative single-destination broadcasts
                # (Dtpb = 1, 2, 4; no absolute core ids needed). One SWDGE
                # queue and one arrival semaphore per round; descriptors are
                # generated now, during the load, and each round's payload
                # (tx0 / s1 / s2) is only read at its trigger. The manually
                # registered replica group inserts the prelude AllGather,
                # which makes the runtime launch all 8 cores together.
                nc._bir_kernel_barrier_sem_replica_groups.append(
                    set(range(N_CORES)))
                rsem = [nc.alloc_semaphore(f"rx_sem{d}") for d in range(3)]
                if SIM_LIGHT:
                    tsem = [nc.alloc_semaphore(f"tx_sem{d}") for d in range(3)]
                else:
                    _ts = nc.alloc_semaphore("tx_sem")
                    tsem = [_ts, _ts, _ts]
                # descriptor preps; Tile's managed trigger path (count=
                # None) orders each round's trigger after its queue's prep
                for q, (d, payload) in enumerate(
                        ((1, tx0), (2, s1), (4, s2))):
                    rd = [None] * 8
                    rd[d] = (0, d)
                    nc.gpsimd.remote_dma_broadcast(
                        rxb[:, q, :], payload[:],
                        remote_sem=rsem[q], local_sem=tsem[q], rdests=rd,
                        queue_num=q,
                    )

            # one-pass partial stats, pipelined with the half-image loads
            for n in range(NB):
                for c in range(NCH):
                    for g in range(7):
                        nc.vector.bn_stats(
                            st6[c][:, (n * 7 + g) * 6:(n * 7 + g + 1) * 6],
                            xt[c][:, n * HW + g * NT: n * HW + (g + 1) * NT],
                        )

            # per-core (mean, var) -> tx0 = [mean/8, E[x^2]/8] per chunk
            mv = main.tile([P, 2 * NCH], F32)
            t_a = main.tile([P, 1], F32)
            t_b = main.tile([P, 1], F32)
            for c in range(NCH):
                nc.vector.bn_aggr(mv[:, 2 * c:2 * c + 2], st6[c][:])
                mean = mv[:, 2 * c:2 * c + 1]
                var = mv[:, 2 * c + 1:2 * c + 2]
                nc.vector.tensor_mul(t_a[:], mean, mean)
                nc.vector.tensor_add(t_b[:], var, t_a[:])
                nc.vector.tensor_scalar_mul(
                    tx0[:, 2 * c:2 * c + 1], mean, 1.0 / N_CORES)
                nc.vector.tensor_scalar_mul(
                    tx0[:, 2 * c + 1:2 * c + 2], t_b[:], 1.0 / N_CORES)

            gs = main.tile([P, 2 * NCH], F32, name="gs")
            if USE_RDMA:
                # order gpsimd behind the vector writes of tx0, then fire all
                # 8 prepared broadcasts; arrivals bump rx_sem by 2 each
                # one critical section holds the whole exchange: entry is
                # gated on the stats (the touch reads tx0), instructions
                # execute in program order inside, and the section is
                # ordered after the descriptor preps because its adds read
                # each prep's out_ap (rxb slots). Remote arrivals bump each
                # round's rsem by 2.
                with tc.tile_critical(name="bfly"):
                    nc.gpsimd.tensor_scalar_mul(tscr[:], tx0[:], 1.0)
                    nc.gpsimd.trigger_dma(1, queue_num=0)
                    nc.gpsimd.wait_ge(rsem[0], 2)
                    nc.gpsimd.memset(dly[:], 0.0)
                    nc.gpsimd.tensor_add(s1[:], tx0[:], rxb[:, 0, :])
                    nc.gpsimd.trigger_dma(1, queue_num=1)
                    nc.gpsimd.wait_ge(rsem[1], 2)
                    nc.gpsimd.memset(dly[:], 0.0)
                    nc.gpsimd.tensor_add(s2[:], s1[:], rxb[:, 1, :])
                    nc.gpsimd.trigger_dma(1, queue_num=2)
                    nc.gpsimd.wait_ge(rsem[2], 2)
                    nc.gpsimd.memset(dly[:], 0.0)
                    nc.gpsimd.tensor_add(gs[:], s2[:], rxb[:, 2, :])
                    if SIM_LIGHT or DBG:
                        nc.gpsimd.tensor_scalar_mul(dbt[:, 0:4], tx0[:], 1.0)
                        nc.gpsimd.tensor_scalar_mul(
                            dbt[:, 4:16],
                            rxb.rearrange("p a b -> p (a b)"), 1.0)
                        nc.gpsimd.tensor_scalar_mul(dbt[:, 16:20], s1[:], 1.0)
                        nc.gpsimd.tensor_scalar_mul(dbt[:, 20:24], s2[:], 1.0)
                        nc.gpsimd.tensor_scalar_mul(dbt[:, 24:28], gs[:], 1.0)
            else:
                cc_in = dram.tile([P, 2 * NCH], F32)
                if USE_AG:
                    # AllGather moves ~half the wire of AllReduce for this
                    # latency-bound 2KB payload; sum the 8 slots locally
                    cc_out = dram.tile([N_CORES, P, 2 * NCH], F32)
                    nc.sync.dma_start(cc_in[:], tx0[:])
                    nc.gpsimd.collective_compute(
                        "AllGather",
                        mybir.AluOpType.bypass,
                        replica_groups=[list(range(N_CORES))],
                        ins=[cc_in[:].opt()],
                        outs=[cc_out[:].opt()],
                    )
                    ga = main.tile([P, N_CORES, 2 * NCH], F32, name="ga")
                    g4 = main.tile([P, 4, 2 * NCH], F32, name="g4")
                    g2 = main.tile([P, 2, 2 * NCH], F32, name="g2")
                    nc.sync.dma_start(
                        ga[:], cc_out.rearrange("w p s -> p w s"))
                    nc.vector.tensor_add(g4[:], ga[:, 0:4, :], ga[:, 4:8, :])
                    nc.vector.tensor_add(g2[:], g4[:, 0:2, :], g4[:, 2:4, :])
                    nc.vector.tensor_add(gs[:], g2[:, 0, :], g2[:, 1, :])
                else:
                    cc_out = dram.tile([P, 2 * NCH], F32)
                    nc.sync.dma_start(cc_in[:], tx0[:])
                    nc.gpsimd.collective_compute(
                        "AllReduce",
                        mybir.AluOpType.add,
                        replica_groups=[list(range(N_CORES))],
                        ins=[cc_in[:].opt()],
                        outs=[cc_out[:].opt()],
                    )
                    nc.sync.dma_start(gs[:], cc_out[:])

            # a = gamma*rsqrt(var+eps), b = beta - mean*a, both chunks at once
            # layouts: gs = [m0,e0,m1,e1]; ab = [a0,a1,b0,b1]
            ab = main.tile([P, 2 * NCH], F32)
            u1 = main.tile([P, NCH], F32)
            u2 = main.tile([P, NCH], F32)
            gsv = gs.rearrange("p (c s) -> p c s", s=2)
            gmean = gsv[:, :, 0]
            ex2 = gsv[:, :, 1]
            parv = parc.rearrange("p (c s) -> p c s", s=3)
            av = ab[:, 0:NCH]
            bv = ab[:, NCH:2 * NCH]
            nc.vector.tensor_mul(u1[:], gmean, gmean)
            nc.vector.tensor_sub(u2[:], ex2, u1[:])          # global var
            nc.vector.tensor_scalar_add(u2[:], u2[:], BN_EPS)
            nc.scalar.activation(u1[:], u2[:], AF.Sqrt)
            nc.vector.reciprocal(u2[:], u1[:])               # rsqrt
            nc.vector.tensor_mul(av, parv[:, :, 0], u2[:])
            nc.vector.tensor_mul(u1[:], gmean, av)
            nc.vector.tensor_sub(bv, parv[:, :, 1], u1[:])

            # normalize + sign -> padded planes; rows split so the first conv
            # block (rows 0..8 of image 0) unblocks as early as possible
            for n in range(NB):
                splits = ((0, 9), (9, 34), (34, H)) if n == 0 else ((0, 34), (34, H))
                for r0, r1 in splits:
                    for c in range(NCH):
                        nc.scalar.activation(
                            xbv[:, c, n, 1 + r0:1 + r1, 1:1 + W],
                            xt[c][:, n * HW + r0 * W:n * HW + r1 * W]
                            .rearrange("p (h w) -> p h w", w=W),
                            AF.Sign,
                            bias=ab[:, NCH + c:NCH + c + 1],
                            scale=ab[:, c:c + 1],
                        )

            # 3x3 binary conv; small leading blocks so matmuls start right
            # after the first sign rows land
            jobs = [] if SIM_LIGHT else [(n, g) for n in range(NB) for g in range(NG)]
            blocks = []
            for o in range(NCH) if not SIM_LIGHT else []:
                sched = [1, 3, BLK, BLK, BLK] if o == 0 else [BLK, BLK, BLK, 4]
                pos = 0
                for step in sched:
                    blocks.append((o, jobs[pos:pos + step]))
                    pos += step
                assert pos == len(jobs)
            for o, blk in blocks:
                pts = [psum.tile([P, NT], F32, name="ps", tag="ps") for _ in blk]
                for t in range(9):
                    ky, kx = divmod(t, 3)
                    w_ap = wb[:, :, (t * NCH + o) * P:(t * NCH + o + 1) * P]
                    for k, (n, g) in enumerate(blk):
                        rhs = xbv[:, :, n, g * RG + ky: g * RG + ky + RG, kx:kx + W]
                        mm = nc.tensor.matmul(
                            pts[k][:], w_ap, rhs,
                            start=(t == 0), stop=(t == 8),
                            perf_mode=mybir.MatmulPerfMode.DoubleRow,
                        )
                        if k > 0:
                            mm.ins.ldweights = False  # reuse loaded weights
                # bias+relu drain on DVE (keeps ScalarE free for sign); one
                # tensor_scalar does (x + bias) then max(.., 0)
                for k, (n, g) in enumerate(blk):
                    ob = outp.tile([P, NT], F32, name="ob", tag="ob")
                    if k % 2 == 0:
                        nc.vector.tensor_scalar(
                            ob[:], pts[k][:],
                            parc[:, 3 * o + 2:3 * o + 3], 0.0,
                            ALU.add, ALU.max,
                        )
                    else:
                        nc.scalar.activation(
                            ob[:], pts[k][:], AF.Relu,
                            bias=parc[:, 3 * o + 2:3 * o + 3])
                    nc.sync.dma_start(
                        ys[n, o * P:(o + 1) * P, g * RG:(g + 1) * RG, :],
                        ob.rearrange("p (h w) -> p h w", w=W),
                    )
            if SIM_LIGHT or DBG:
                nc.sync.dma_start(dbg[0:P], dbt[:])
    nc.compile()
    return nc


def _get_nc():
    if "nc" not in _CACHE:
        _CACHE["nc"] = _build_nc()
    return _CACHE["nc"]


def _prep_inputs(x, gamma, beta, weight, bias):
    wsign = np.sign(weight.astype(np.float32))
    # [p(ci_in), j(ci_chunk), (tap, o_chunk, co_in)]
    wT = (
        wsign.reshape(NCH, P, NCH, P, 3, 3)      # o, m, c, p, ky, kx
        .transpose(3, 2, 4, 5, 0, 1)             # p, c, ky, kx, o, m
        .reshape(P, NCH, 9 * NCH * P)
        .astype(mybir.dt.np(FP8))
    )
    par = np.stack(
        [gamma.astype(np.float32), beta.astype(np.float32), bias.astype(np.float32)],
        axis=-1,
    ).reshape(NCH, P, 3)
    x = np.ascontiguousarray(x, dtype=np.float32)
    in_maps = [
        {"xs": x[j * NB:(j + 1) * NB], "wt": wT, "par": par}
        for j in range(N_CORES)
    ]
    return in_maps


def _run(x, gamma, beta, weight, bias, trace=False):
    nc = _get_nc()
    in_maps = _prep_inputs(x, gamma, beta, weight, bias)
    res = bass_utils.run_bass_kernel_spmd(
        nc, in_maps, core_ids=list(range(N_CORES)), trace=trace
    )
    out = np.concatenate([res.results[j]["ys"] for j in range(N_CORES)], axis=0)
    return out, res


def kernel(x, gamma, beta, weight, bias):
    out, _ = _run(x, gamma, beta, weight, bias, trace=False)
    return out


# revision 37
# speedup vs baseline: 1.0154x; 1.0154x over previous
"""Binary conv + BN(train) + ReLU fused Trainium2 SPMD kernel.

Reference computation (NCHW, x:(32,256,56,56) f32):
    mean/var over (N,H,W) per channel; xn = (x-mean)*rsqrt(var+eps)*gamma+beta
    xb = sign(xn); wb = sign(W); y = relu(conv3x3(xb, wb, pad=1) + bias)

Strategy: data-parallel over batch across 8 NeuronCores (4 images each).
Per-core partial BN stats (bn_stats/bn_aggr on DVE, pipelined with the x
load) are exchanged via direct core-to-core RDMA broadcasts (XOR-relative
destinations, one 16B/partition payload per core; remote semaphores count
arrivals) instead of a CC AllReduce, cutting ~50us of collective latency.
Normalize+sign runs as scalar-engine activations Sign(a*x+b) writing fp8
into zero-padded 58x58 planes; the 3x3 conv is 9 accumulating DoubleRow
fp8 matmuls (K=256 via the paired-row mode) per 128x448 output tile with
weights loaded once per (block, tap); bias+relu drains on the vector
engine (tensor_scalar add+max), keeping the scalar engine free for sign.
Dummy matmuls gated on stats-readiness warm the PE clock (HAM) during the
exchange. Sign values are exact in fp8 and PSUM accumulates in fp32, so
the binarized conv is exact.
"""

import sys

for _p in ("/opt/trn_rl_repo", "/root/.axon_site/_ro/trn_rl_repo"):
    if _p not in sys.path:
        sys.path.append(_p)

import numpy as np

import concourse.bass as bass
from concourse.bass import ds
import concourse.mybir as mybir
import concourse.tile as tile
from concourse import bacc, bass_utils

F32 = mybir.dt.float32
FP8 = mybir.dt.float8e4
AF = mybir.ActivationFunctionType
ALU = mybir.AluOpType

N_CORES = 8
NB = 4          # images per core
C = 256
P = 128         # partitions / chunk size
NCH = 2         # channel chunks (ci and co)
H = W = 56
HW = H * W      # 3136
PH = PW = 58    # padded plane
PSZ = PH * PW   # 3364
RG = 8          # output rows per psum tile
NG = H // RG    # 7 row groups
NT = RG * W     # 448 columns per matmul
BN_EPS = 1e-5
BLK = 8         # psum tiles in flight per weight-reuse block

USE_RDMA = False  # direct RDMA stats exchange instead of CC AllReduce
USE_AG = True      # AllGather + local sum instead of AllReduce
SIM_LIGHT = False  # skip conv+warmup (sim can't model 4D DoubleRow matmuls)
DBG = False        # dump exchange tiles to the dbg output
DK_WARM = 40      # dummy matmuls to warm the PE while ab/sign run

_CACHE = {}


def _emit_warmup(nc, psum, wdum, wb):
    """PE warmup: zero matmuls gated on wdum (written at exchange-trigger
    time) keep the HAM activity window busy so the real conv starts at full
    clock. Results land in a pool psum tile that is never read."""
    dums = psum.tile([P, NT], F32, name="dums", tag="ps")
    for _ in range(DK_WARM):
        nc.tensor.matmul(
            dums[:], wdum[:], wb[:, :, 0:NT],
            start=True, stop=True,
            perf_mode=mybir.MatmulPerfMode.DoubleRow,
        )


def _build_nc():
    nc = bacc.Bacc("TRN2", target_bir_lowering=False, debug=False,
                   num_devices=N_CORES, num_swdge_queues=3)
    xs = nc.dram_tensor("xs", [NB, C, H, W], F32, kind="ExternalInput")
    wt = nc.dram_tensor("wt", [P, NCH, 9 * NCH * P], FP8, kind="ExternalInput")
    par = nc.dram_tensor("par", [NCH, P, 3], F32, kind="ExternalInput")
    ys = nc.dram_tensor("ys", [NB, C, H, W], F32, kind="ExternalOutput")
    dbg = (nc.dram_tensor("dbg", [P, 28], F32, kind="ExternalOutput")
           if (SIM_LIGHT or DBG) else None)

    with tile.TileContext(nc) as tc:
        with (
            tc.tile_pool(name="main", bufs=1) as main,
            tc.tile_pool(name="outp", bufs=4) as outp,
            tc.tile_pool(name="psum", bufs=8, space="PSUM") as psum,
            tc.tile_pool(name="dram", bufs=1, space="DRAM") as dram,
        ):
            xt = [main.tile([P, NB * HW], F32, name=f"xt{c}") for c in range(NCH)]
            # sign planes: [p, ci_chunk, image, padded 58x58] (chunk dim = fp8
            # DoubleRow pair dim)
            xball = main.tile([P, NCH, NB * PSZ], FP8, name="xball")
            xbv = xball.rearrange("p j (n h w) -> p j n h w", n=NB, h=PH)
            wb = main.tile([P, NCH, 9 * NCH * P], FP8, name="wb")
            parc = main.tile([P, 3 * NCH], F32, name="parc")  # [gamma,beta,bias] x chunk
            st6 = [main.tile([P, NB * 7 * 6], F32, name=f"st6{c}") for c in range(NCH)]
            tx0 = main.tile([P, 4], F32, name="tx0")      # [m0,e0,m1,e1]/8 payload
            rxb = main.tile([P, 3, 4], F32, name="rxb")   # butterfly recv slots
            s1 = main.tile([P, 4], F32, name="s1")
            s2 = main.tile([P, 4], F32, name="s2")
            tscr = main.tile([P, 4], F32, name="tscr")
            dbt = main.tile([P, 28], F32, name="dbt")
            wdum = main.tile([P, NCH, P], FP8, name="wdum")
            scr_a = main.tile([P, 1], F32, name="scr_a")
            scr_b = main.tile([P, 1], F32, name="scr_b")

            # load x (channels on partitions): chunk 0 from ScalarE's HWDGE
            # queue, chunk 1 from Sync's, so ring configs overlap and the load
            # starts as early as either engine clears its preamble
            HH = HW // 2
            for n in range(NB):
                for h in range(2):
                    nc.scalar.dma_start(
                        xt[0][:, n * HW + h * HH:n * HW + (h + 1) * HH],
                        xs[n, 0:P].rearrange("p h w -> p (h w)")[:, h * HH:(h + 1) * HH],
                    )
                    nc.sync.dma_start(
                        xt[1][:, n * HW + h * HH:n * HW + (h + 1) * HH],
                        xs[n, P:2 * P].rearrange("p h w -> p (h w)")[:, h * HH:(h + 1) * HH],
                    )
            nc.scalar.dma_start(wb[:], wt[:])
            nc.sync.dma_start(
                parc.rearrange("p (c s) -> p c s", s=3),
                par.rearrange("c p s -> p c s"),
            )

            # activation-table preload: pull sqrt_and_friends (covers Sqrt/
            # Sign/Relu) into the scalar engine during the x load so no
            # ACT_TABLE_LOAD lands on the post-exchange critical path
            nc.gpsimd.memset(scr_a[:], 1.0)
            nc.scalar.activation(scr_b[:], scr_a[:], AF.Sqrt)
            nc.scalar.activation(scr_b[:], scr_a[:], AF.Sign)
            nc.scalar.activation(scr_b[:], scr_a[:], AF.Relu)

            # zero only the pad borders of the sign planes (GpSimd; interior
            # is fully overwritten by the Sign activation)
            nc.gpsimd.memset(tx0[:], 0.0)
            for c in range(NCH):
                for n in range(NB):
                    nc.gpsimd.memset(xbv[:, c, n, 0, :], 0.0)
                    nc.gpsimd.memset(xbv[:, c, n, PH - 1, :], 0.0)
                    nc.gpsimd.memset(xbv[:, c, n, 1:PH - 1, 0], 0.0)
                    nc.gpsimd.memset(xbv[:, c, n, 1:PH - 1, PW - 1], 0.0)

            if USE_RDMA:
                # hypercube butterfly all-reduce of the 2KB stats payload in
                # 3 rounds of XOR-relative single-destination broadcasts
                # (Dtpb = 1, 2, 4; no absolute core ids needed). One SWDGE
                # queue and one arrival semaphore per round; descriptors are
                # generated now, during the load, and each round's payload
                # (tx0 / s1 / s2) is only read at its trigger. The manually
                # registered replica group inserts the prelude AllGather,
                # which makes the runtime launch all 8 cores together.
                nc._bir_kernel_barrier_sem_replica_groups.append(
                    set(range(N_CORES)))
                rsem = [nc.alloc_semaphore(f"rx_sem{d}") for d in range(3)]
                if SIM_LIGHT:
                    tsem = [nc.alloc_semaphore(f"tx_sem{d}") for d in range(3)]
                else:
                    _ts = nc.alloc_semaphore("tx_sem")
                    tsem = [_ts, _ts, _ts]
                # descriptor preps; Tile's managed trigger path (count=
                # None) orders each round's trigger after its queue's prep
                for q, (d, payload) in enumerate(
                        ((1, tx0), (2, s1), (4, s2))):
                    rd = [None] * 8
                    rd[d] = (0, d)
                    nc.gpsimd.remote_dma_broadcast(
                        rxb[:, q, :], payload[:],
                        remote_sem=rsem[q], local_sem=tsem[q], rdests=rd,
                        queue_num=q,
                    )

            # one-pass partial stats, pipelined with the half-image loads
            for n in range(NB):
                for c in range(NCH):
                    for g in range(7):
                        nc.vector.bn_stats(
                            st6[c][:, (n * 7 + g) * 6:(n * 7 + g + 1) * 6],
                            xt[c][:, n * HW + g * NT: n * HW + (g + 1) * NT],
                        )

            # per-core (mean, var) -> tx0 = [mean/8, E[x^2]/8] per chunk
            mv = main.tile([P, 2 * NCH], F32)
            t_a = main.tile([P, 1], F32)
            t_b = main.tile([P, 1], F32)
            for c in range(NCH):
                nc.vector.bn_aggr(mv[:, 2 * c:2 * c + 2], st6[c][:])
                mean = mv[:, 2 * c:2 * c + 1]
                var = mv[:, 2 * c + 1:2 * c + 2]
                nc.vector.tensor_mul(t_a[:], mean, mean)
                nc.vector.tensor_add(t_b[:], var, t_a[:])
                nc.vector.tensor_scalar_mul(
                    tx0[:, 2 * c:2 * c + 1], mean, 1.0 / N_CORES)
                nc.vector.tensor_scalar_mul(
                    tx0[:, 2 * c + 1:2 * c + 2], t_b[:], 1.0 / N_CORES)

            gs = main.tile([P, 2 * NCH], F32, name="gs")
            if USE_RDMA:
                # order gpsimd behind the vector writes of tx0, then fire all
                # 8 prepared broadcasts; arrivals bump rx_sem by 2 each
                # one critical section holds the whole exchange: entry is
                # gated on the stats (the touch reads tx0), instructions
                # execute in program order inside, and the section is
                # ordered after the descriptor preps because its adds read
                # each prep's out_ap (rxb slots). Remote arrivals bump each
                # round's rsem by 2.
                with tc.tile_critical(name="bfly"):
                    nc.gpsimd.tensor_scalar_mul(tscr[:], tx0[:], 1.0)
                    nc.gpsimd.trigger_dma(1, queue_num=0)
                    nc.gpsimd.wait_ge(rsem[0], 2)
                    nc.gpsimd.memset(dly[:], 0.0)
                    nc.gpsimd.tensor_add(s1[:], tx0[:], rxb[:, 0, :])
                    nc.gpsimd.trigger_dma(1, queue_num=1)
                    nc.gpsimd.wait_ge(rsem[1], 2)
                    nc.gpsimd.memset(dly[:], 0.0)
                    nc.gpsimd.tensor_add(s2[:], s1[:], rxb[:, 1, :])
                    nc.gpsimd.trigger_dma(1, queue_num=2)
                    nc.gpsimd.wait_ge(rsem[2], 2)
                    nc.gpsimd.memset(dly[:], 0.0)
                    nc.gpsimd.tensor_add(gs[:], s2[:], rxb[:, 2, :])
                    if SIM_LIGHT or DBG:
                        nc.gpsimd.tensor_scalar_mul(dbt[:, 0:4], tx0[:], 1.0)
                        nc.gpsimd.tensor_scalar_mul(
                            dbt[:, 4:16],
                            rxb.rearrange("p a b -> p (a b)"), 1.0)
                        nc.gpsimd.tensor_scalar_mul(dbt[:, 16:20], s1[:], 1.0)
                        nc.gpsimd.tensor_scalar_mul(dbt[:, 20:24], s2[:], 1.0)
                        nc.gpsimd.tensor_scalar_mul(dbt[:, 24:28], gs[:], 1.0)
            else:
                cc_in = dram.tile([P, 2 * NCH], F32)
                if USE_AG:
                    # AllGather moves ~half the wire of AllReduce for this
                    # latency-bound 2KB payload; sum the 8 slots locally
                    cc_out = dram.tile([N_CORES, P, 2 * NCH], F32)
                    nc.sync.dma_start(cc_in[:], tx0[:])
                    nc.gpsimd.collective_compute(
                        "AllGather",
                        mybir.AluOpType.bypass,
                        replica_groups=[list(range(N_CORES))],
                        ins=[cc_in[:].opt()],
                        outs=[cc_out[:].opt()],
                    )
                    ga = main.tile([P, N_CORES, 2 * NCH], F32, name="ga")
                    g4 = main.tile([P, 4, 2 * NCH], F32, name="g4")
                    g2 = main.tile([P, 2, 2 * NCH], F32, name="g2")
                    nc.sync.dma_start(
                        ga[:], cc_out.rearrange("w p s -> p w s"))
                    nc.vector.tensor_add(g4[:], ga[:, 0:4, :], ga[:, 4:8, :])
                    nc.vector.tensor_add(g2[:], g4[:, 0:2, :], g4[:, 2:4, :])
                    nc.vector.tensor_add(gs[:], g2[:, 0, :], g2[:, 1, :])
                    # release the PE warmup right as the global stats land,
                    # so the HAM clock is hot when the first real matmul
                    # issues ~7us later
                    nc.vector.tensor_scalar_mul(wdum[:, 0, 0:4], gs[:], 0.0)
                    if not SIM_LIGHT:
                        _emit_warmup(nc, psum, wdum, wb)
                else:
                    cc_out = dram.tile([P, 2 * NCH], F32)
                    nc.sync.dma_start(cc_in[:], tx0[:])
                    nc.gpsimd.collective_compute(
                        "AllReduce",
                        mybir.AluOpType.add,
                        replica_groups=[list(range(N_CORES))],
                        ins=[cc_in[:].opt()],
                        outs=[cc_out[:].opt()],
                    )
                    nc.sync.dma_start(gs[:], cc_out[:])

            # a = gamma*rsqrt(var+eps), b = beta - mean*a, both chunks at once
            # layouts: gs = [m0,e0,m1,e1]; ab = [a0,a1,b0,b1]
            ab = main.tile([P, 2 * NCH], F32)
            u1 = main.tile([P, NCH], F32)
            u2 = main.tile([P, NCH], F32)
            gsv = gs.rearrange("p (c s) -> p c s", s=2)
            gmean = gsv[:, :, 0]
            ex2 = gsv[:, :, 1]
            parv = parc.rearrange("p (c s) -> p c s", s=3)
            av = ab[:, 0:NCH]
            bv = ab[:, NCH:2 * NCH]
            nc.vector.tensor_mul(u1[:], gmean, gmean)
            nc.vector.tensor_sub(u2[:], ex2, u1[:])          # global var
            nc.vector.tensor_scalar_add(u2[:], u2[:], BN_EPS)
            nc.scalar.activation(u1[:], u2[:], AF.Sqrt)
            nc.vector.reciprocal(u2[:], u1[:])               # rsqrt
            nc.vector.tensor_mul(av, parv[:, :, 0], u2[:])
            nc.vector.tensor_mul(u1[:], gmean, av)
            nc.vector.tensor_sub(bv, parv[:, :, 1], u1[:])

            # normalize + sign -> padded planes; rows split so the first conv
            # block (rows 0..8 of image 0) unblocks as early as possible
            for n in range(NB):
                splits = ((0, 9), (9, 34), (34, H)) if n == 0 else ((0, 34), (34, H))
                for r0, r1 in splits:
                    for c in range(NCH):
                        nc.scalar.activation(
                            xbv[:, c, n, 1 + r0:1 + r1, 1:1 + W],
                            xt[c][:, n * HW + r0 * W:n * HW + r1 * W]
                            .rearrange("p (h w) -> p h w", w=W),
                            AF.Sign,
                            bias=ab[:, NCH + c:NCH + c + 1],
                            scale=ab[:, c:c + 1],
                        )

            # 3x3 binary conv; small leading blocks so matmuls start right
            # after the first sign rows land
            jobs = [] if SIM_LIGHT else [(n, g) for n in range(NB) for g in range(NG)]
            blocks = []
            for o in range(NCH) if not SIM_LIGHT else []:
                sched = [1, 3, BLK, BLK, BLK] if o == 0 else [BLK, BLK, BLK, 4]
                pos = 0
                for step in sched:
                    blocks.append((o, jobs[pos:pos + step]))
                    pos += step
                assert pos == len(jobs)
            for o, blk in blocks:
                pts = [psum.tile([P, NT], F32, name="ps", tag="ps") for _ in blk]
                for t in range(9):
                    ky, kx = divmod(t, 3)
                    w_ap = wb[:, :, (t * NCH + o) * P:(t * NCH + o + 1) * P]
                    for k, (n, g) in enumerate(blk):
                        rhs = xbv[:, :, n, g * RG + ky: g * RG + ky + RG, kx:kx + W]
                        mm = nc.tensor.matmul(
                            pts[k][:], w_ap, rhs,
                            start=(t == 0), stop=(t == 8),
                            perf_mode=mybir.MatmulPerfMode.DoubleRow,
                        )
                        if k > 0:
                            mm.ins.ldweights = False  # reuse loaded weights
                # bias+relu drain on DVE (keeps ScalarE free for sign); one
                # tensor_scalar does (x + bias) then max(.., 0)
                for k, (n, g) in enumerate(blk):
                    ob = outp.tile([P, NT], F32, name="ob", tag="ob")
                    if k % 2 == 0:
                        nc.vector.tensor_scalar(
                            ob[:], pts[k][:],
                            parc[:, 3 * o + 2:3 * o + 3], 0.0,
                            ALU.add, ALU.max,
                        )
                    else:
                        nc.scalar.activation(
                            ob[:], pts[k][:], AF.Relu,
                            bias=parc[:, 3 * o + 2:3 * o + 3])
                    nc.sync.dma_start(
                        ys[n, o * P:(o + 1) * P, g * RG:(g + 1) * RG, :],
                        ob.rearrange("p (h w) -> p h w", w=W),
                    )
            if SIM_LIGHT or DBG:
                nc.sync.dma_start(dbg[0:P], dbt[:])
    nc.compile()
    return nc


def _get_nc():
    if "nc" not in _CACHE:
        _CACHE["nc"] = _build_nc()
    return _CACHE["nc"]


def _prep_inputs(x, gamma, beta, weight, bias):
    wsign = np.sign(weight.astype(np.float32))
    # [p(ci_in), j(ci_chunk), (tap, o_chunk, co_in)]
    wT = (
        wsign.reshape(NCH, P, NCH, P, 3, 3)      # o, m, c, p, ky, kx
        .transpose(3, 2, 4, 5, 0, 1)             # p, c, ky, kx, o, m
        .reshape(P, NCH, 9 * NCH * P)
        .astype(mybir.dt.np(FP8))
    )
    par = np.stack(
        [gamma.astype(np.float32), beta.astype(np.float32), bias.astype(np.float32)],
        axis=-1,
    ).reshape(NCH, P, 3)
    x = np.ascontiguousarray(x, dtype=np.float32)
    in_maps = [
        {"xs": x[j * NB:(j + 1) * NB], "wt": wT, "par": par}
        for j in range(N_CORES)
    ]
    return in_maps


def _run(x, gamma, beta, weight, bias, trace=False):
    nc = _get_nc()
    in_maps = _prep_inputs(x, gamma, beta, weight, bias)
    res = bass_utils.run_bass_kernel_spmd(
        nc, in_maps, core_ids=list(range(N_CORES)), trace=trace
    )
    out = np.concatenate([res.results[j]["ys"] for j in range(N_CORES)], axis=0)
    return out, res


def kernel(x, gamma, beta, weight, bias):
    out, _ = _run(x, gamma, beta, weight, bias, trace=False)
    return out


# revision 38
# speedup vs baseline: 1.0696x; 1.0534x over previous
"""Binary conv + BN(train) + ReLU fused Trainium2 SPMD kernel.

Reference computation (NCHW, x:(32,256,56,56) f32):
    mean/var over (N,H,W) per channel; xn = (x-mean)*rsqrt(var+eps)*gamma+beta
    xb = sign(xn); wb = sign(W); y = relu(conv3x3(xb, wb, pad=1) + bias)

Strategy: data-parallel over batch across 8 NeuronCores (4 images each).
The x load is split into half-image transfers across two HWDGE queues
(ScalarE + Sync) so per-core BN stats (bn_stats/bn_aggr on DVE) pipeline
behind the load at half-image grain and trail load-end by only a few us.
Global stats use a 2KB-per-rank CC AllGather (about half the wire and
latency of an AllReduce at this latency-bound size) followed by a local
8-slot sum on DVE. Normalize+sign runs as scalar-engine activations
Sign(a*x+b) writing fp8 into zero-padded 58x58 planes, first rows of
image 0 first so the conv unblocks early; the 3x3 conv is 9 accumulating
DoubleRow fp8 matmuls (K=256 via the paired-row mode) per 128x448 output
tile. Zero dummy matmuls released when the gathered stats land keep the
PE HAM clock warm through the ab/sign chain, so the conv starts at
2.4GHz. Bias+relu drains alternate between DVE (tensor_scalar add+max)
and ScalarE, and stores stream from Sync. Sign values are exact in fp8
and PSUM accumulates in fp32, so the binarized conv is exact; results
match the f32 reference bit-for-bit in testing.

(USE_RDMA retains an experimental direct core-to-core exchange via
XOR-relative remote_dma broadcasts; it is off because remote semaphores
can fire before the payload is visible in the receiver's SBUF.)
"""

import sys

for _p in ("/opt/trn_rl_repo", "/root/.axon_site/_ro/trn_rl_repo"):
    if _p not in sys.path:
        sys.path.append(_p)

import numpy as np

import concourse.bass as bass
from concourse.bass import ds
import concourse.mybir as mybir
import concourse.tile as tile
from concourse import bacc, bass_utils

F32 = mybir.dt.float32
FP8 = mybir.dt.float8e4
AF = mybir.ActivationFunctionType
ALU = mybir.AluOpType

N_CORES = 8
NB = 4          # images per core
C = 256
P = 128         # partitions / chunk size
NCH = 2         # channel chunks (ci and co)
H = W = 56
HW = H * W      # 3136
PH = PW = 58    # padded plane
PSZ = PH * PW   # 3364
RG = 8          # output rows per psum tile
NG = H // RG    # 7 row groups
NT = RG * W     # 448 columns per matmul
BN_EPS = 1e-5
BLK = 8         # psum tiles in flight per weight-reuse block

USE_RDMA = False  # direct RDMA stats exchange instead of CC AllReduce
USE_AG = True      # AllGather + local sum instead of AllReduce
SIM_LIGHT = False  # skip conv+warmup (sim can't model 4D DoubleRow matmuls)
DBG = False        # dump exchange tiles to the dbg output
DK_WARM = 40      # dummy matmuls to warm the PE while ab/sign run

_CACHE = {}


def _emit_warmup(nc, psum, wdum, wb):
    """PE warmup: zero matmuls gated on wdum (written at exchange-trigger
    time) keep the HAM activity window busy so the real conv starts at full
    clock. Results land in a pool psum tile that is never read."""
    dums = psum.tile([P, NT], F32, name="dums", tag="ps")
    for _ in range(DK_WARM):
        nc.tensor.matmul(
            dums[:], wdum[:], wb[:, :, 0:NT],
            start=True, stop=True,
            perf_mode=mybir.MatmulPerfMode.DoubleRow,
        )


def _build_nc():
    nc = bacc.Bacc("TRN2", target_bir_lowering=False, debug=False,
                   num_devices=N_CORES, num_swdge_queues=3)
    xs = nc.dram_tensor("xs", [NB, C, H, W], F32, kind="ExternalInput")
    wt = nc.dram_tensor("wt", [P, NCH, 9 * NCH * P], FP8, kind="ExternalInput")
    par = nc.dram_tensor("par", [NCH, P, 3], F32, kind="ExternalInput")
    ys = nc.dram_tensor("ys", [NB, C, H, W], F32, kind="ExternalOutput")
    dbg = (nc.dram_tensor("dbg", [P, 28], F32, kind="ExternalOutput")
           if (SIM_LIGHT or DBG) else None)

    with tile.TileContext(nc) as tc:
        with (
            tc.tile_pool(name="main", bufs=1) as main,
            tc.tile_pool(name="outp", bufs=4) as outp,
            tc.tile_pool(name="psum", bufs=8, space="PSUM") as psum,
            tc.tile_pool(name="dram", bufs=1, space="DRAM") as dram,
        ):
            xt = [main.tile([P, NB * HW], F32, name=f"xt{c}") for c in range(NCH)]
            # sign planes: [p, ci_chunk, image, padded 58x58] (chunk dim = fp8
            # DoubleRow pair dim)
            xball = main.tile([P, NCH, NB * PSZ], FP8, name="xball")
            xbv = xball.rearrange("p j (n h w) -> p j n h w", n=NB, h=PH)
            wb = main.tile([P, NCH, 9 * NCH * P], FP8, name="wb")
            parc = main.tile([P, 3 * NCH], F32, name="parc")  # [gamma,beta,bias] x chunk
            st6 = [main.tile([P, NB * 7 * 6], F32, name=f"st6{c}") for c in range(NCH)]
            tx0 = main.tile([P, 4], F32, name="tx0")      # [m0,e0,m1,e1]/8 payload
            rxb = main.tile([P, 3, 4], F32, name="rxb")   # butterfly recv slots
            s1 = main.tile([P, 4], F32, name="s1")
            s2 = main.tile([P, 4], F32, name="s2")
            tscr = main.tile([P, 4], F32, name="tscr")
            dbt = main.tile([P, 28], F32, name="dbt")
            wdum = main.tile([P, NCH, P], FP8, name="wdum")
            scr_a = main.tile([P, 1], F32, name="scr_a")
            scr_b = main.tile([P, 1], F32, name="scr_b")

            # load x (channels on partitions): chunk 0 from ScalarE's HWDGE
            # queue, chunk 1 from Sync's, so ring configs overlap and the load
            # starts as early as either engine clears its preamble
            HH = HW // 2
            for n in range(NB):
                for h in range(2):
                    nc.scalar.dma_start(
                        xt[0][:, n * HW + h * HH:n * HW + (h + 1) * HH],
                        xs[n, 0:P].rearrange("p h w -> p (h w)")[:, h * HH:(h + 1) * HH],
                    )
                    nc.sync.dma_start(
                        xt[1][:, n * HW + h * HH:n * HW + (h + 1) * HH],
                        xs[n, P:2 * P].rearrange("p h w -> p (h w)")[:, h * HH:(h + 1) * HH],
                    )
            nc.scalar.dma_start(wb[:], wt[:])
            nc.sync.dma_start(
                parc.rearrange("p (c s) -> p c s", s=3),
                par.rearrange("c p s -> p c s"),
            )

            # activation-table preload: pull sqrt_and_friends (covers Sqrt/
            # Sign/Relu) into the scalar engine during the x load so no
            # ACT_TABLE_LOAD lands on the post-exchange critical path
            nc.gpsimd.memset(scr_a[:], 1.0)
            nc.scalar.activation(scr_b[:], scr_a[:], AF.Sqrt)
            nc.scalar.activation(scr_b[:], scr_a[:], AF.Sign)
            nc.scalar.activation(scr_b[:], scr_a[:], AF.Relu)

            # zero only the pad borders of the sign planes (GpSimd; interior
            # is fully overwritten by the Sign activation)
            nc.gpsimd.memset(tx0[:], 0.0)
            for c in range(NCH):
                for n in range(NB):
                    nc.gpsimd.memset(xbv[:, c, n, 0, :], 0.0)
                    nc.gpsimd.memset(xbv[:, c, n, PH - 1, :], 0.0)
                    nc.gpsimd.memset(xbv[:, c, n, 1:PH - 1, 0], 0.0)
                    nc.gpsimd.memset(xbv[:, c, n, 1:PH - 1, PW - 1], 0.0)

            if USE_RDMA:
                # hypercube butterfly all-reduce of the 2KB stats payload in
                # 3 rounds of XOR-relative single-destination broadcasts
                # (Dtpb = 1, 2, 4; no absolute core ids needed). One SWDGE
                # queue and one arrival semaphore per round; descriptors are
                # generated now, during the load, and each round's payload
                # (tx0 / s1 / s2) is only read at its trigger. The manually
                # registered replica group inserts the prelude AllGather,
                # which makes the runtime launch all 8 cores together.
                nc._bir_kernel_barrier_sem_replica_groups.append(
                    set(range(N_CORES)))
                rsem = [nc.alloc_semaphore(f"rx_sem{d}") for d in range(3)]
                if SIM_LIGHT:
                    tsem = [nc.alloc_semaphore(f"tx_sem{d}") for d in range(3)]
                else:
                    _ts = nc.alloc_semaphore("tx_sem")
                    tsem = [_ts, _ts, _ts]
                # descriptor preps; Tile's managed trigger path (count=
                # None) orders each round's trigger after its queue's prep
                for q, (d, payload) in enumerate(
                        ((1, tx0), (2, s1), (4, s2))):
                    rd = [None] * 8
                    rd[d] = (0, d)
                    nc.gpsimd.remote_dma_broadcast(
                        rxb[:, q, :], payload[:],
                        remote_sem=rsem[q], local_sem=tsem[q], rdests=rd,
                        queue_num=q,
                    )

            # one-pass partial stats, pipelined with the half-image loads
            for n in range(NB):
                for c in range(NCH):
                    for g in range(7):
                        nc.vector.bn_stats(
                            st6[c][:, (n * 7 + g) * 6:(n * 7 + g + 1) * 6],
                            xt[c][:, n * HW + g * NT: n * HW + (g + 1) * NT],
                        )

            # per-core (mean, var) -> tx0 = [mean/8, E[x^2]/8] per chunk
            mv = main.tile([P, 2 * NCH], F32)
            t_a = main.tile([P, 1], F32)
            t_b = main.tile([P, 1], F32)
            for c in range(NCH):
                nc.vector.bn_aggr(mv[:, 2 * c:2 * c + 2], st6[c][:])
                mean = mv[:, 2 * c:2 * c + 1]
                var = mv[:, 2 * c + 1:2 * c + 2]
                nc.vector.tensor_mul(t_a[:], mean, mean)
                nc.vector.tensor_add(t_b[:], var, t_a[:])
                nc.vector.tensor_scalar_mul(
                    tx0[:, 2 * c:2 * c + 1], mean, 1.0 / N_CORES)
                nc.vector.tensor_scalar_mul(
                    tx0[:, 2 * c + 1:2 * c + 2], t_b[:], 1.0 / N_CORES)

            gs = main.tile([P, 2 * NCH], F32, name="gs")
            if USE_RDMA:
                # order gpsimd behind the vector writes of tx0, then fire all
                # 8 prepared broadcasts; arrivals bump rx_sem by 2 each
                # one critical section holds the whole exchange: entry is
                # gated on the stats (the touch reads tx0), instructions
                # execute in program order inside, and the section is
                # ordered after the descriptor preps because its adds read
                # each prep's out_ap (rxb slots). Remote arrivals bump each
                # round's rsem by 2.
                with tc.tile_critical(name="bfly"):
                    nc.gpsimd.tensor_scalar_mul(tscr[:], tx0[:], 1.0)
                    nc.gpsimd.trigger_dma(1, queue_num=0)
                    nc.gpsimd.wait_ge(rsem[0], 2)
                    nc.gpsimd.memset(dly[:], 0.0)
                    nc.gpsimd.tensor_add(s1[:], tx0[:], rxb[:, 0, :])
                    nc.gpsimd.trigger_dma(1, queue_num=1)
                    nc.gpsimd.wait_ge(rsem[1], 2)
                    nc.gpsimd.memset(dly[:], 0.0)
                    nc.gpsimd.tensor_add(s2[:], s1[:], rxb[:, 1, :])
                    nc.gpsimd.trigger_dma(1, queue_num=2)
                    nc.gpsimd.wait_ge(rsem[2], 2)
                    nc.gpsimd.memset(dly[:], 0.0)
                    nc.gpsimd.tensor_add(gs[:], s2[:], rxb[:, 2, :])
                    if SIM_LIGHT or DBG:
                        nc.gpsimd.tensor_scalar_mul(dbt[:, 0:4], tx0[:], 1.0)
                        nc.gpsimd.tensor_scalar_mul(
                            dbt[:, 4:16],
                            rxb.rearrange("p a b -> p (a b)"), 1.0)
                        nc.gpsimd.tensor_scalar_mul(dbt[:, 16:20], s1[:], 1.0)
                        nc.gpsimd.tensor_scalar_mul(dbt[:, 20:24], s2[:], 1.0)
                        nc.gpsimd.tensor_scalar_mul(dbt[:, 24:28], gs[:], 1.0)
            else:
                cc_in = dram.tile([P, 2 * NCH], F32)
                if USE_AG:
                    # AllGather moves ~half the wire of AllReduce for this
                    # latency-bound 2KB payload; sum the 8 slots locally
                    cc_out = dram.tile([N_CORES, P, 2 * NCH], F32)
                    nc.sync.dma_start(cc_in[:], tx0[:])
                    nc.gpsimd.collective_compute(
                        "AllGather",
                        mybir.AluOpType.bypass,
                        replica_groups=[list(range(N_CORES))],
                        ins=[cc_in[:].opt()],
                        outs=[cc_out[:].opt()],
                    )
                    ga = main.tile([P, N_CORES, 2 * NCH], F32, name="ga")
                    g4 = main.tile([P, 4, 2 * NCH], F32, name="g4")
                    g2 = main.tile([P, 2, 2 * NCH], F32, name="g2")
                    nc.sync.dma_start(
                        ga[:], cc_out.rearrange("w p s -> p w s"))
                    nc.vector.tensor_add(g4[:], ga[:, 0:4, :], ga[:, 4:8, :])
                    nc.vector.tensor_add(g2[:], g4[:, 0:2, :], g4[:, 2:4, :])
                    nc.vector.tensor_add(gs[:], g2[:, 0, :], g2[:, 1, :])
                    # release the PE warmup right as the global stats land,
                    # so the HAM clock is hot when the first real matmul
                    # issues ~7us later
                    nc.vector.tensor_scalar_mul(wdum[:, 0, 0:4], gs[:], 0.0)
                    if not SIM_LIGHT:
                        _emit_warmup(nc, psum, wdum, wb)
                else:
                    cc_out = dram.tile([P, 2 * NCH], F32)
                    nc.sync.dma_start(cc_in[:], tx0[:])
                    nc.gpsimd.collective_compute(
                        "AllReduce",
                        mybir.AluOpType.add,
                        replica_groups=[list(range(N_CORES))],
                        ins=[cc_in[:].opt()],
                        outs=[cc_out[:].opt()],
                    )
                    nc.sync.dma_start(gs[:], cc_out[:])

            # a = gamma*rsqrt(var+eps), b = beta - mean*a, both chunks at once
            # layouts: gs = [m0,e0,m1,e1]; ab = [a0,a1,b0,b1]
            ab = main.tile([P, 2 * NCH], F32)
            u1 = main.tile([P, NCH], F32)
            u2 = main.tile([P, NCH], F32)
            gsv = gs.rearrange("p (c s) -> p c s", s=2)
            gmean = gsv[:, :, 0]
            ex2 = gsv[:, :, 1]
            parv = parc.rearrange("p (c s) -> p c s", s=3)
            av = ab[:, 0:NCH]
            bv = ab[:, NCH:2 * NCH]
            nc.vector.tensor_mul(u1[:], gmean, gmean)
            nc.vector.tensor_sub(u2[:], ex2, u1[:])          # global var
            nc.vector.tensor_scalar_add(u2[:], u2[:], BN_EPS)
            nc.scalar.activation(u1[:], u2[:], AF.Sqrt)
            nc.vector.reciprocal(u2[:], u1[:])               # rsqrt
            nc.vector.tensor_mul(av, parv[:, :, 0], u2[:])
            nc.vector.tensor_mul(u1[:], gmean, av)
            nc.vector.tensor_sub(bv, parv[:, :, 1], u1[:])

            # normalize + sign -> padded planes; rows split so the first conv
            # block (rows 0..8 of image 0) unblocks as early as possible
            for n in range(NB):
                splits = ((0, 9), (9, 34), (34, H)) if n == 0 else ((0, 34), (34, H))
                for r0, r1 in splits:
                    for c in range(NCH):
                        nc.scalar.activation(
                            xbv[:, c, n, 1 + r0:1 + r1, 1:1 + W],
                            xt[c][:, n * HW + r0 * W:n * HW + r1 * W]
                            .rearrange("p (h w) -> p h w", w=W),
                            AF.Sign,
                            bias=ab[:, NCH + c:NCH + c + 1],
                            scale=ab[:, c:c + 1],
                        )

            # 3x3 binary conv; small leading blocks so matmuls start right
            # after the first sign rows land
            jobs = [] if SIM_LIGHT else [(n, g) for n in range(NB) for g in range(NG)]
            blocks = []
            for o in range(NCH) if not SIM_LIGHT else []:
                sched = [1, 3, BLK, BLK, BLK] if o == 0 else [BLK, BLK, BLK, 4]
                pos = 0
                for step in sched:
                    blocks.append((o, jobs[pos:pos + step]))
                    pos += step
                assert pos == len(jobs)
            for o, blk in blocks:
                pts = [psum.tile([P, NT], F32, name="ps", tag="ps") for _ in blk]
                for t in range(9):
                    ky, kx = divmod(t, 3)
                    w_ap = wb[:, :, (t * NCH + o) * P:(t * NCH + o + 1) * P]
                    for k, (n, g) in enumerate(blk):
                        rhs = xbv[:, :, n, g * RG + ky: g * RG + ky + RG, kx:kx + W]
                        mm = nc.tensor.matmul(
                            pts[k][:], w_ap, rhs,
                            start=(t == 0), stop=(t == 8),
                            perf_mode=mybir.MatmulPerfMode.DoubleRow,
                        )
                        if k > 0:
                            mm.ins.ldweights = False  # reuse loaded weights
                # bias+relu drain on DVE (keeps ScalarE free for sign); one
                # tensor_scalar does (x + bias) then max(.., 0)
                for k, (n, g) in enumerate(blk):
                    ob = outp.tile([P, NT], F32, name="ob", tag="ob")
                    if k % 2 == 0:
                        nc.vector.tensor_scalar(
                            ob[:], pts[k][:],
                            parc[:, 3 * o + 2:3 * o + 3], 0.0,
                            ALU.add, ALU.max,
                        )
                    else:
                        nc.scalar.activation(
                            ob[:], pts[k][:], AF.Relu,
                            bias=parc[:, 3 * o + 2:3 * o + 3])
                    nc.sync.dma_start(
                        ys[n, o * P:(o + 1) * P, g * RG:(g + 1) * RG, :],
                        ob.rearrange("p (h w) -> p h w", w=W),
                    )
            if SIM_LIGHT or DBG:
                nc.sync.dma_start(dbg[0:P], dbt[:])
    nc.compile()
    return nc


def _get_nc():
    if "nc" not in _CACHE:
        _CACHE["nc"] = _build_nc()
    return _CACHE["nc"]


def _prep_inputs(x, gamma, beta, weight, bias):
    wsign = np.sign(weight.astype(np.float32))
    # [p(ci_in), j(ci_chunk), (tap, o_chunk, co_in)]
    wT = (
        wsign.reshape(NCH, P, NCH, P, 3, 3)      # o, m, c, p, ky, kx
        .transpose(3, 2, 4, 5, 0, 1)             # p, c, ky, kx, o, m
        .reshape(P, NCH, 9 * NCH * P)
        .astype(mybir.dt.np(FP8))
    )
    par = np.stack(
        [gamma.astype(np.float32), beta.astype(np.float32), bias.astype(np.float32)],
        axis=-1,
    ).reshape(NCH, P, 3)
    x = np.ascontiguousarray(x, dtype=np.float32)
    in_maps = [
        {"xs": x[j * NB:(j + 1) * NB], "wt": wT, "par": par}
        for j in range(N_CORES)
    ]
    return in_maps


def _run(x, gamma, beta, weight, bias, trace=False):
    nc = _get_nc()
    in_maps = _prep_inputs(x, gamma, beta, weight, bias)
    res = bass_utils.run_bass_kernel_spmd(
        nc, in_maps, core_ids=list(range(N_CORES)), trace=trace
    )
    out = np.concatenate([res.results[j]["ys"] for j in range(N_CORES)], axis=0)
    return out, res


def kernel(x, gamma, beta, weight, bias):
    out, _ = _run(x, gamma, beta, weight, bias, trace=False)
    return out
